# revision 14
# baseline (speedup 1.0000x reference)
"""Trainium2 Bass kernel for the gated mixed-norm + top-k-mask module.

Contract: kernel(**inputs) takes FULL inputs (B=8,C=512,L=16384,H=256),
shards batch across 8 NeuronCores (1 sample per core), runs one SPMD Bass
program, returns the FULL [8,512,16384] float32 output.

Algorithm per core (one sample, x/c are [512, 16384] fp32):
  Phase A: x streams in via casting SWDGE DMAs (f32 DRAM -> fp16 SBUF)
           directly into a resident 16 MiB copy (x read from HBM exactly
           once); bn_stats per channel (mean/var) on DVE.
  Phase B: c streams on the SP HWDGE ring in f32; cast to fp8e4m3 with
           fp32 `cond` row-sums fused via accum_out — split 2 groups on
           ACT (Copy activation) / 2 on DVE so neither engine backlogs;
           gate MLP matmuls on PE in fp8 DoubleRow perf mode; Relu/
           Sigmoid on ACT with accum_out giving per-channel g_mix sums.
  Params:  layer stats via ones-matmul partition reduction + law of total
           variance; tiny fp32 MLP for gamma/beta; fold everything into a
           per-channel affine y = a*x + b.
  Phase C: imp[c] = sum_l |a*x+b| from resident fp16 x, split 5:3
           ACT (Abs activation + accum, ~7.3us/unit) : DVE (affine +
           abs-reduce, ~10.9us/unit) so both finish in ~37us.
  Mask:    rank_i = #{j: imp_j > imp_i} via PE transpose + partition
           broadcast + is_gt accum; keep rank < 358; fold mask into a,b.
  Phase D: y = a'*x + b' from resident fp16 x on DVE; output DMAs
           alternate between the two HWDGE rings (SP / ACT) for ~440 GB/s.
"""

import sys

for _p in ("/opt/trn_rl_repo",):
    if _p not in sys.path:
        sys.path.insert(0, _p)

from contextlib import ExitStack

import numpy as np

import concourse.bacc as bacc
import concourse.tile as tile
from concourse import mybir
from concourse.bass_utils import run_bass_kernel_spmd
from concourse.masks import make_identity

F32 = mybir.dt.float32
F16 = mybir.dt.float16
BF16 = mybir.dt.bfloat16
FP8 = mybir.dt.float8e4

B = 8
C = 512
H = 256
P = 128
G = C // P          # 4 channel groups of 128
MH = H // P         # 2 h-halves of 128
EPS = 1e-3
KEEP = max(1, int(C * 0.7))   # 358


def build_program(L=16384, lt_a=2048, lt_b=1024, lt_c=8192, lt_d=4096):
    """Build the SPMD Bass program for one core (one sample)."""
    lt_a, lt_b, lt_c, lt_d = (min(t, L) for t in (lt_a, lt_b, lt_c, lt_d))
    nt_a = L // lt_a
    nt_b = L // lt_b
    nt_c = L // lt_c
    nt_d = L // lt_d

    nc = bacc.Bacc("TRN2", target_bir_lowering=False, debug=False)

    x_d = nc.dram_tensor("x", [C, L], F32, kind="ExternalInput")
    c_d = nc.dram_tensor("c", [C, L], F32, kind="ExternalInput")
    w1t_d = nc.dram_tensor("w1t", [C, H], F32, kind="ExternalInput")    # gate_w1.T
    b1_d = nc.dram_tensor("b1", [H], F32, kind="ExternalInput")
    w2t_d = nc.dram_tensor("w2t", [H, C], F32, kind="ExternalInput")    # gate_w2.T
    b2_d = nc.dram_tensor("b2", [C], F32, kind="ExternalInput")
    mw1t_d = nc.dram_tensor("mw1t", [C, H], F32, kind="ExternalInput")  # mlp_w1.T
    mb1_d = nc.dram_tensor("mb1", [H], F32, kind="ExternalInput")
    mw2t_d = nc.dram_tensor("mw2t", [H, 2 * C], F32, kind="ExternalInput")  # mlp_w2.T
    mb2_d = nc.dram_tensor("mb2", [2 * C], F32, kind="ExternalInput")
    y_d = nc.dram_tensor("y", [C, L], F32, kind="ExternalOutput")

    # channel ch = g*128 + p  <->  [p, g]
    xr = x_d.ap().rearrange("(g p) l -> p g l", g=G)
    cr = c_d.ap().rearrange("(g p) l -> p g l", g=G)
    yr = y_d.ap().rearrange("(g p) l -> p g l", g=G)
    w1tr = w1t_d.ap().rearrange("(k p) m -> p k m", k=G)     # [128, 4, 256]
    w2tr = w2t_d.ap().rearrange("(k p) m -> p k m", k=MH)    # [128, 2, 512]
    mw1tr = mw1t_d.ap().rearrange("(k p) m -> p k m", k=G)   # [128, 4, 256]
    mw2tr = mw2t_d.ap().rearrange("(k p) m -> p k m", k=MH)  # [128, 2, 1024]
    b1r = b1_d.ap().rearrange("(m p) -> p m", m=MH)          # [128, 2]
    b2r = b2_d.ap().rearrange("(m p) -> p m", m=G)           # [128, 4]
    mb1r = mb1_d.ap().rearrange("(m p) -> p m", m=MH)        # [128, 2]
    mb2r = mb2_d.ap().rearrange("(m p) -> p m", m=2 * G)     # [128, 8]

    with tile.TileContext(nc) as tc, ExitStack() as top:
        big = top.enter_context(tc.tile_pool(name="big", bufs=1))

        # ---- persistent tiles ----
        xres = big.tile([P, G, L], F16)                      # resident fp16 x
        stats_parts = big.tile([P, G, nt_a, lt_a // 512, 6], F32)
        gmix_parts = big.tile([P, G, nt_b], F32)
        cond_parts = big.tile([P, G, nt_b], F32)
        imp_parts = big.tile([P, G, nt_c], F32)
        mv = big.tile([P, G, 2], F32)                        # per-channel mean/var
        w1_sb = big.tile([P, G, H], FP8)
        w2_sb = big.tile([P, MH, C], FP8)
        b1_sb = big.tile([P, MH], F32)
        b2_sb = big.tile([P, G], F32)
        mw1_sb = big.tile([P, G, H], F32)
        mw2_sb = big.tile([P, MH, 2 * C], F32)
        mb1_sb = big.tile([P, MH], F32)
        mb2_sb = big.tile([P, 2 * G], F32)
        eps_sb = big.tile([P, 1], F32)
        ones_sb = big.tile([P, 1], F32)
        # small result tiles
        a_sb = big.tile([P, G], F32)        # (1+gamma)/sigma
        bb_sb = big.tile([P, G], F32)       # beta - mu*a
        sig_c = big.tile([P, G], F32)
        gmix = big.tile([P, G], F32)
        cond = big.tile([P, G], F32)
        imp = big.tile([P, G], F32)
        rank = big.tile([P, G], F32)
        mask = big.tile([P, G], F32)
        gb_sb = big.tile([P, 2 * G], F32)   # gamma | beta
        h2_sb = big.tile([P, MH], F32)
        musig_l = big.tile([1, 2], F32)     # [mu_l, sigma_l] on partition 0
        musig_b = big.tile([P, 2], F32)     # broadcast to all partitions
        lscr = big.tile([1, 8], F32)        # partition-0 scratch
        sums3 = big.tile([1, 3], F32)
        vec_sb = big.tile([P, G, 3], F32)   # [mu, var, mu^2] per channel

        nc.vector.memset(eps_sb, EPS)
        nc.vector.memset(ones_sb, 1.0)

        # ---- load + prep weights (all small; on the ACT HWDGE ring so the
        # c stream owns the SP ring from instruction 0) ----
        with tc.tile_pool(name="wstage", bufs=1) as wst:
            w1_f32 = wst.tile([P, G, H], F32)
            w2_f32 = wst.tile([P, MH, C], F32)
            nc.scalar.dma_start(out=w1_f32, in_=w1tr)
            nc.scalar.dma_start(out=w2_f32, in_=w2tr)
            nc.vector.tensor_copy(out=w1_sb, in_=w1_f32)
            nc.vector.tensor_copy(out=w2_sb, in_=w2_f32)
            nc.scalar.dma_start(out=b1_sb, in_=b1r)
            nc.scalar.dma_start(out=b2_sb, in_=b2r)
            nc.scalar.dma_start(out=mw1_sb, in_=mw1tr)
            nc.scalar.dma_start(out=mw2_sb, in_=mw2tr)
            nc.scalar.dma_start(out=mb1_sb, in_=mb1r)
            nc.scalar.dma_start(out=mb2_sb, in_=mb2r)

        # ---- phases B (c: gate matmuls) and A (x: stats), interleaved ----
        with ExitStack() as ab:
            cpool = ab.enter_context(tc.tile_pool(name="cpool", bufs=3))
            work = ab.enter_context(tc.tile_pool(name="work", bufs=2))
            ps_h = ab.enter_context(tc.tile_pool(name="ps_h", bufs=2, space="PSUM"))
            ps_g = ab.enter_context(tc.tile_pool(name="ps_g", bufs=2, space="PSUM"))

            def emit_a(i):
                g, ti = divmod(i, nt_a)
                sl = slice(ti * lt_a, (ti + 1) * lt_a)
                # casting DMA (SWDGE): f32 DRAM -> fp16 resident SBUF in one
                # step; no staging tile, no ACT copy.
                nc.gpsimd.dma_start(out=xres[:, g, sl], in_=xr[:, g, sl])
                xv = xres[:, g, sl].rearrange("p (s f) -> p s f", f=512)
                for s in range(lt_a // 512):
                    nc.vector.bn_stats(
                        out=stats_parts[:, g, ti, s], in_=xv[:, s]
                    )

            def emit_front(t):
                # cast + mm1 + relu for tile t.  The f32->fp8 cast with
                # fused cond row-sum is split 2 groups on ACT / 2 on DVE:
                # DVE alone (bn_stats + 4 casts) was the A+B straggler.
                sl = slice(t * lt_b, (t + 1) * lt_b)
                ct = cpool.tile([P, G, lt_b], F32, tag="c")
                nc.sync.dma_start(out=ct, in_=cr[:, :, sl])
                cbf = work.tile([P, G, lt_b], FP8, tag="cbf", bufs=1)
                for g in range(G):
                    if g < 2:
                        nc.vector.tensor_scalar(
                            out=cbf[:, g], in0=ct[:, g],
                            scalar1=1.0, scalar2=None,
                            op0=mybir.AluOpType.mult,
                            op1=mybir.AluOpType.add,
                            accum_out=cond_parts[:, g, t : t + 1],
                        )
                    else:
                        nc.scalar.activation(
                            out=cbf[:, g], in_=ct[:, g],
                            func=mybir.ActivationFunctionType.Copy,
                            accum_out=cond_parts[:, g, t : t + 1],
                        )
                hbf = work.tile([P, MH, lt_b], FP8, tag="hbf", bufs=1)
                for m in range(MH):
                    hps = ps_h.tile([P, lt_b], F32, tag="hps")
                    for n in range(lt_b // 512):
                        nsl = slice(n * 512, (n + 1) * 512)
                        for kk in range(0, G, 2):
                            nc.tensor.matmul(
                                hps[:, nsl],
                                lhsT=w1_sb[:, kk : kk + 2, m * P : (m + 1) * P],
                                rhs=cbf[:, kk : kk + 2, nsl],
                                start=(kk == 0), stop=(kk == G - 2),
                                perf_mode=mybir.MatmulPerfMode.DoubleRow,
                            )
                    nc.scalar.activation(
                        out=hbf[:, m], in_=hps,
                        func=mybir.ActivationFunctionType.Relu,
                        bias=b1_sb[:, m : m + 1], scale=1.0,
                    )
                return hbf

            def emit_back(t, hbf):
                # mm2 + sigmoid for tile t
                for g in range(G):
                    gps = ps_g.tile([P, lt_b], F32, tag="gps")
                    for n in range(lt_b // 512):
                        nsl = slice(n * 512, (n + 1) * 512)
                        nc.tensor.matmul(
                            gps[:, nsl],
                            lhsT=w2_sb[:, 0:MH, g * P : (g + 1) * P],
                            rhs=hbf[:, 0:MH, nsl],
                            start=True, stop=True,
                            perf_mode=mybir.MatmulPerfMode.DoubleRow,
                        )
                    gscr = work.tile([P, lt_b], FP8, tag="gscr", bufs=1)
                    nc.scalar.activation(
                        out=gscr, in_=gps,
                        func=mybir.ActivationFunctionType.Sigmoid,
                        bias=b2_sb[:, g : g + 1], scale=1.0,
                        accum_out=gmix_parts[:, g, t : t + 1],
                    )

            # Phases B and A interleaved so neither queues fully behind
            # the other (priority follows emission order).
            n_x_tiles = G * nt_a
            for t in range(nt_b):
                hbf = emit_front(t)
                emit_back(t, hbf)
                for i in range(
                    t * n_x_tiles // nt_b, (t + 1) * n_x_tiles // nt_b
                ):
                    emit_a(i)

        # ---- stats aggregation + tiny MLP + per-channel affine params ----
        with tc.tile_pool(name="ps_t", bufs=1, space="PSUM") as ps_t:
            # per-channel mean/var
            for g in range(G):
                nc.vector.bn_aggr(out=mv[:, g], in_=stats_parts[:, g])

            # layer stats: reduce over all 512 channels with a ones-matmul
            nc.vector.tensor_copy(out=vec_sb[:, :, 0:2], in_=mv)
            nc.vector.tensor_mul(
                out=vec_sb[:, :, 2], in0=mv[:, :, 0], in1=mv[:, :, 0]
            )
            lps = ps_t.tile([1, G * 3], F32, tag="lps")
            nc.tensor.matmul(
                lps, lhsT=ones_sb, rhs=vec_sb.rearrange("p g k -> p (g k)"),
                start=True, stop=True,
            )
            l_sb = big.tile([1, G * 3], F32)
            nc.vector.tensor_copy(out=l_sb, in_=lps)
            nc.vector.reduce_sum(
                out=sums3,
                in_=l_sb.rearrange("p (g k) -> p k g", g=G),
                axis=mybir.AxisListType.X,
            )
            # mu_l = S_mu/C ; var_l = (S_var + S_mu2)/C - mu_l^2
            nc.vector.tensor_add(
                out=lscr[:, 0:1], in0=sums3[:, 1:2], in1=sums3[:, 2:3]
            )
            nc.vector.tensor_scalar_mul(
                out=lscr[:, 0:1], in0=lscr[:, 0:1], scalar1=1.0 / C
            )
            nc.vector.tensor_scalar_mul(
                out=lscr[:, 1:2], in0=sums3[:, 0:1], scalar1=1.0 / C
            )
            nc.vector.tensor_mul(
                out=lscr[:, 2:3], in0=lscr[:, 1:2], in1=lscr[:, 1:2]
            )
            nc.vector.tensor_sub(
                out=lscr[:, 3:4], in0=lscr[:, 0:1], in1=lscr[:, 2:3]
            )
            nc.scalar.activation(
                out=musig_l[:, 1:2], in_=lscr[:, 3:4],
                func=mybir.ActivationFunctionType.Sqrt,
                bias=eps_sb[0:1], scale=1.0,
            )
            nc.vector.tensor_copy(out=musig_l[:, 0:1], in_=lscr[:, 1:2])
            nc.gpsimd.partition_broadcast(musig_b, musig_l)

            # sigma_c = sqrt(var_c + eps)
            nc.scalar.activation(
                out=sig_c, in_=mv[:, :, 1],
                func=mybir.ActivationFunctionType.Sqrt,
                bias=eps_sb, scale=1.0,
            )
            # g_mix, cond
            nc.vector.reduce_sum(out=gmix, in_=gmix_parts, axis=mybir.AxisListType.X)
            nc.vector.tensor_scalar_mul(out=gmix, in0=gmix, scalar1=1.0 / L)
            nc.vector.reduce_sum(out=cond, in_=cond_parts, axis=mybir.AxisListType.X)
            nc.vector.tensor_scalar_mul(out=cond, in0=cond, scalar1=1.0 / L)

            # tiny MLP: gamma/beta = mw2 @ relu(mw1 @ cond + mb1) + mb2
            h2ps = ps_t.tile([P, MH], F32, tag="h2ps")
            for m in range(MH):
                for k in range(G):
                    nc.tensor.matmul(
                        h2ps[:, m : m + 1],
                        lhsT=mw1_sb[:, k, m * P : (m + 1) * P],
                        rhs=cond[:, k : k + 1],
                        start=(k == 0), stop=(k == G - 1),
                    )
            for m in range(MH):
                nc.scalar.activation(
                    out=h2_sb[:, m : m + 1], in_=h2ps[:, m : m + 1],
                    func=mybir.ActivationFunctionType.Relu,
                    bias=mb1_sb[:, m : m + 1], scale=1.0,
                )
            gbps = ps_t.tile([P, 2 * G], F32, tag="gbps")
            for mg in range(2 * G):
                for k in range(MH):
                    nc.tensor.matmul(
                        gbps[:, mg : mg + 1],
                        lhsT=mw2_sb[:, k, mg * P : (mg + 1) * P],
                        rhs=h2_sb[:, k : k + 1],
                        start=(k == 0), stop=(k == MH - 1),
                    )
            nc.vector.tensor_add(out=gb_sb, in0=gbps, in1=mb2_sb)

            # mu = mu_l + gmix*(mu_c - mu_l); sigma = sig_l + gmix*(sig_c - sig_l)
            mu_t = big.tile([P, G], F32)
            sg_t = big.tile([P, G], F32)
            nc.vector.tensor_scalar(
                out=mu_t, in0=mv[:, :, 0], scalar1=musig_b[:, 0:1], scalar2=None,
                op0=mybir.AluOpType.subtract,
            )
            nc.vector.tensor_mul(out=mu_t, in0=mu_t, in1=gmix)
            nc.vector.tensor_scalar(
                out=mu_t, in0=mu_t, scalar1=musig_b[:, 0:1], scalar2=None,
                op0=mybir.AluOpType.add,
            )
            nc.vector.tensor_scalar(
                out=sg_t, in0=sig_c, scalar1=musig_b[:, 1:2], scalar2=None,
                op0=mybir.AluOpType.subtract,
            )
            nc.vector.tensor_mul(out=sg_t, in0=sg_t, in1=gmix)
            nc.vector.tensor_scalar(
                out=sg_t, in0=sg_t, scalar1=musig_b[:, 1:2], scalar2=None,
                op0=mybir.AluOpType.add,
            )
            # a = (1+gamma)/sigma ; b = beta - mu*a
            rs_t = big.tile([P, G], F32)
            nc.vector.reciprocal(out=rs_t, in_=sg_t)
            nc.vector.tensor_scalar(
                out=a_sb, in0=gb_sb[:, 0:G], scalar1=1.0, scalar2=None,
                op0=mybir.AluOpType.add,
            )
            nc.vector.tensor_mul(out=a_sb, in0=a_sb, in1=rs_t)
            nc.vector.tensor_mul(out=bb_sb, in0=mu_t, in1=a_sb)
            nc.vector.tensor_sub(out=bb_sb, in0=gb_sb[:, G : 2 * G], in1=bb_sb)

        # ---- phase C: imp = sum |a*x + b| over L, from resident fp16 x ----
        with ExitStack() as cs:
            scp = cs.enter_context(tc.tile_pool(name="scp", bufs=2))
            ps_m = cs.enter_context(tc.tile_pool(name="ps_m", bufs=1, space="PSUM"))
            ident_sb = scp.tile([P, P], F32, bufs=1)
            impF = scp.tile([1, C], F32, bufs=1)
            impB = scp.tile([P, C], F32, bufs=1)
            make_identity(nc, ident_sb)
            # Split by measured per-unit cost: ACT (Abs+accum) ~7.3us,
            # DVE (affine + abs-reduce) ~10.9us -> 5 : 3 of 8 units.
            for u, (t, g) in enumerate(
                (t, g) for t in range(nt_c) for g in range(G)
            ):
                sl = slice(t * lt_c, (t + 1) * lt_c)
                scr = scp.tile([P, lt_c], F16, tag="scr")
                if u % 8 in (0, 2, 3, 5, 7):
                    # ACT: imp accum via |a*x + b| activation
                    nc.scalar.activation(
                        out=scr, in_=xres[:, g, sl],
                        func=mybir.ActivationFunctionType.Abs,
                        bias=bb_sb[:, g : g + 1], scale=a_sb[:, g : g + 1],
                        accum_out=imp_parts[:, g, t : t + 1],
                    )
                else:
                    # DVE: affine then abs-reduce (offloads ACT)
                    nc.vector.tensor_scalar(
                        out=scr, in0=xres[:, g, sl],
                        scalar1=a_sb[:, g : g + 1],
                        scalar2=bb_sb[:, g : g + 1],
                        op0=mybir.AluOpType.mult, op1=mybir.AluOpType.add,
                    )
                    nc.vector.tensor_reduce(
                        out=imp_parts[:, g, t : t + 1], in_=scr,
                        axis=mybir.AxisListType.X, op=mybir.AluOpType.add,
                        apply_absolute_value=True,
                    )
            nc.vector.reduce_sum(out=imp, in_=imp_parts, axis=mybir.AxisListType.X)

            # rank_i = #{j : imp_j > imp_i};  mask = rank < KEEP
            tps = ps_m.tile([1, C], F32, tag="tps")
            for g in range(G):
                nc.tensor.transpose(
                    tps[:, g * P : (g + 1) * P], imp[:, g : g + 1], ident_sb
                )
            nc.vector.tensor_copy(out=impF, in_=tps)
            nc.gpsimd.partition_broadcast(impB, impF)
            cscr = scp.tile([P, C], F32, tag="cscr")
            for g in range(G):
                nc.vector.tensor_scalar(
                    out=cscr, in0=impB,
                    scalar1=imp[:, g : g + 1], scalar2=None,
                    op0=mybir.AluOpType.is_gt,
                    op1=mybir.AluOpType.add,
                    accum_out=rank[:, g : g + 1],
                )
            nc.vector.tensor_scalar(
                out=mask, in0=rank, scalar1=float(KEEP), scalar2=None,
                op0=mybir.AluOpType.is_lt,
            )
            nc.vector.tensor_mul(out=a_sb, in0=a_sb, in1=mask)
            nc.vector.tensor_mul(out=bb_sb, in0=bb_sb, in1=mask)

        # ---- phase D: y = a'*x + b' ----
        with tc.tile_pool(name="od", bufs=3) as od:
            for i, (g, t) in enumerate(
                (g, t) for t in range(nt_d) for g in range(G)
            ):
                sl = slice(t * lt_d, (t + 1) * lt_d)
                ot = od.tile([P, lt_d], F32, tag="ot")
                nc.vector.tensor_scalar(
                    out=ot, in0=xres[:, g, sl],
                    scalar1=a_sb[:, g : g + 1], scalar2=bb_sb[:, g : g + 1],
                    op0=mybir.AluOpType.mult, op1=mybir.AluOpType.add,
                )
                # alternate between the two HWDGE rings
                ring = nc.sync if i % 2 == 0 else nc.scalar
                ring.dma_start(out=yr[:, g, sl], in_=ot)

    nc.compile()
    return nc


_CACHED = {}


def _get_program(L=16384):
    if L not in _CACHED:
        _CACHED[L] = build_program(L=L)
    return _CACHED[L]


def make_in_maps(inputs, n_cores=B):
    f = np.float32
    w1t = np.ascontiguousarray(np.asarray(inputs["gate_w1"], dtype=f).T)
    w2t = np.ascontiguousarray(np.asarray(inputs["gate_w2"], dtype=f).T)
    mw1t = np.ascontiguousarray(np.asarray(inputs["mlp_w1"], dtype=f).T)
    mw2t = np.ascontiguousarray(np.asarray(inputs["mlp_w2"], dtype=f).T)
    b1 = np.ascontiguousarray(np.asarray(inputs["gate_b1"], dtype=f))
    b2 = np.ascontiguousarray(np.asarray(inputs["gate_b2"], dtype=f))
    mb1 = np.ascontiguousarray(np.asarray(inputs["mlp_b1"], dtype=f))
    mb2 = np.ascontiguousarray(np.asarray(inputs["mlp_b2"], dtype=f))
    x = np.asarray(inputs["x16"], dtype=f)
    c = np.asarray(inputs["c16"], dtype=f)
    maps = []
    for b in range(n_cores):
        maps.append({
            "x": np.ascontiguousarray(x[b]),
            "c": np.ascontiguousarray(c[b]),
            "w1t": w1t, "b1": b1, "w2t": w2t, "b2": b2,
            "mw1t": mw1t, "mb1": mb1, "mw2t": mw2t, "mb2": mb2,
        })
    return maps


def run(inputs, trace=False, **kw):
    """Run on 8 cores; returns (output [8,C,L], BassKernelResults)."""
    nc = _get_program()
    in_maps = make_in_maps(inputs)
    res = run_bass_kernel_spmd(nc, in_maps, list(range(B)), trace=trace, **kw)
    out = np.stack([res.results[i]["y"] for i in range(B)], axis=0)
    return out, res


def kernel(**inputs):
    out, _ = run(inputs)
    return out.astype(np.asarray(inputs["x16"]).dtype, copy=False)


# revision 15
# speedup vs baseline: 1.1241x; 1.1241x over previous
"""Trainium2 Bass kernel for the gated mixed-norm + top-k-mask module.

Contract: kernel(**inputs) takes FULL inputs (B=8,C=512,L=16384,H=256),
shards batch across 8 NeuronCores (1 sample per core), runs one SPMD Bass
program, returns the FULL [8,512,16384] float32 output.

Algorithm per core (one sample, x/c are [512, 16384] fp32):
  Phase A: x streams in via casting SWDGE DMAs (f32 DRAM -> fp16 SBUF)
           directly into a resident 16 MiB copy (x read from HBM exactly
           once); bn_stats per channel (mean/var) on DVE.
  Phase B: c streams on the SP HWDGE ring in f32; cast to fp8e4m3 with
           fp32 `cond` row-sums fused via accum_out — split 2 groups on
           ACT (Copy activation) / 2 on DVE so neither engine backlogs;
           gate MLP matmuls on PE in fp8 DoubleRow perf mode; Relu/
           Sigmoid on ACT with accum_out giving per-channel g_mix sums.
  Params:  layer stats via ones-matmul partition reduction + law of total
           variance; tiny fp32 MLP for gamma/beta; fold everything into a
           per-channel affine y = a*x + b.
  Phase C: imp[c] = sum_l |a*x+b| from resident fp16 x, split 5:3
           ACT (Abs activation + accum, ~7.3us/unit) : DVE (affine +
           abs-reduce, ~10.9us/unit) so both finish in ~37us.
  Mask:    rank_i = #{j: imp_j > imp_i} via PE transpose + partition
           broadcast + is_gt accum; keep rank < 358; fold mask into a,b.
  Phase D: y = a'*x + b' from resident fp16 x on DVE; output DMAs
           alternate between the two HWDGE rings (SP / ACT) for ~440 GB/s.
"""

import sys

for _p in ("/opt/trn_rl_repo",):
    if _p not in sys.path:
        sys.path.insert(0, _p)

from contextlib import ExitStack

import numpy as np

import concourse.bacc as bacc
import concourse.tile as tile
from concourse import mybir
from concourse.bass_utils import run_bass_kernel_spmd
from concourse.masks import make_identity

F32 = mybir.dt.float32
F16 = mybir.dt.float16
BF16 = mybir.dt.bfloat16
FP8 = mybir.dt.float8e4

B = 8
C = 512
H = 256
P = 128
G = C // P          # 4 channel groups of 128
MH = H // P         # 2 h-halves of 128
EPS = 1e-3
KEEP = max(1, int(C * 0.7))   # 358


def build_program(L=16384, lt_a=2048, lt_b=1024, lt_c=8192, lt_d=4096):
    """Build the SPMD Bass program for one core (one sample)."""
    lt_a, lt_b, lt_c, lt_d = (min(t, L) for t in (lt_a, lt_b, lt_c, lt_d))
    nt_a = L // lt_a
    nt_b = L // lt_b
    nt_c = L // lt_c
    nt_d = L // lt_d

    nc = bacc.Bacc("TRN2", target_bir_lowering=False, debug=False)

    x_d = nc.dram_tensor("x", [C, L], F32, kind="ExternalInput")
    c_d = nc.dram_tensor("c", [C, L], F32, kind="ExternalInput")
    w1t_d = nc.dram_tensor("w1t", [C, H], F32, kind="ExternalInput")    # gate_w1.T
    b1_d = nc.dram_tensor("b1", [H], F32, kind="ExternalInput")
    w2t_d = nc.dram_tensor("w2t", [H, C], F32, kind="ExternalInput")    # gate_w2.T
    b2_d = nc.dram_tensor("b2", [C], F32, kind="ExternalInput")
    mw1t_d = nc.dram_tensor("mw1t", [C, H], F32, kind="ExternalInput")  # mlp_w1.T
    mb1_d = nc.dram_tensor("mb1", [H], F32, kind="ExternalInput")
    mw2t_d = nc.dram_tensor("mw2t", [H, 2 * C], F32, kind="ExternalInput")  # mlp_w2.T
    mb2_d = nc.dram_tensor("mb2", [2 * C], F32, kind="ExternalInput")
    y_d = nc.dram_tensor("y", [C, L], F32, kind="ExternalOutput")

    # channel ch = g*128 + p  <->  [p, g]
    xr = x_d.ap().rearrange("(g p) l -> p g l", g=G)
    cr = c_d.ap().rearrange("(g p) l -> p g l", g=G)
    yr = y_d.ap().rearrange("(g p) l -> p g l", g=G)
    w1tr = w1t_d.ap().rearrange("(k p) m -> p k m", k=G)     # [128, 4, 256]
    w2tr = w2t_d.ap().rearrange("(k p) m -> p k m", k=MH)    # [128, 2, 512]
    mw1tr = mw1t_d.ap().rearrange("(k p) m -> p k m", k=G)   # [128, 4, 256]
    mw2tr = mw2t_d.ap().rearrange("(k p) m -> p k m", k=MH)  # [128, 2, 1024]
    b1r = b1_d.ap().rearrange("(m p) -> p m", m=MH)          # [128, 2]
    b2r = b2_d.ap().rearrange("(m p) -> p m", m=G)           # [128, 4]
    mb1r = mb1_d.ap().rearrange("(m p) -> p m", m=MH)        # [128, 2]
    mb2r = mb2_d.ap().rearrange("(m p) -> p m", m=2 * G)     # [128, 8]

    with tile.TileContext(nc) as tc, ExitStack() as top:
        big = top.enter_context(tc.tile_pool(name="big", bufs=1))

        # ---- persistent tiles ----
        xres = big.tile([P, G, L], F16)                      # resident fp16 x
        stats_parts = big.tile([P, G, nt_a, lt_a // 512, 6], F32)
        gmix_parts = big.tile([P, G, nt_b], F32)
        cond_parts = big.tile([P, G, nt_b], F32)
        imp_parts = big.tile([P, G, nt_c], F32)
        mv = big.tile([P, G, 2], F32)                        # per-channel mean/var
        w1_sb = big.tile([P, G, H], FP8)
        w2_sb = big.tile([P, MH, C], FP8)
        b1_sb = big.tile([P, MH], F32)
        b2_sb = big.tile([P, G], F32)
        mw1_sb = big.tile([P, G, H], F32)
        mw2_sb = big.tile([P, MH, 2 * C], F32)
        mb1_sb = big.tile([P, MH], F32)
        mb2_sb = big.tile([P, 2 * G], F32)
        eps_sb = big.tile([P, 1], F32)
        ones_sb = big.tile([P, 1], F32)
        # small result tiles
        a_sb = big.tile([P, G], F32)        # (1+gamma)/sigma
        bb_sb = big.tile([P, G], F32)       # beta - mu*a
        sig_c = big.tile([P, G], F32)
        gmix = big.tile([P, G], F32)
        cond = big.tile([P, G], F32)
        imp = big.tile([P, G], F32)
        rank = big.tile([P, G], F32)
        mask = big.tile([P, G], F32)
        gb_sb = big.tile([P, 2 * G], F32)   # gamma | beta
        h2_sb = big.tile([P, MH], F32)
        musig_l = big.tile([1, 2], F32)     # [mu_l, sigma_l] on partition 0
        musig_b = big.tile([P, 2], F32)     # broadcast to all partitions
        lscr = big.tile([1, 8], F32)        # partition-0 scratch
        sums3 = big.tile([1, 3], F32)
        vec_sb = big.tile([P, G, 3], F32)   # [mu, var, mu^2] per channel

        nc.vector.memset(eps_sb, EPS)
        nc.vector.memset(ones_sb, 1.0)

        # ---- load + prep weights (all small; on the ACT HWDGE ring so the
        # c stream owns the SP ring from instruction 0) ----
        with tc.tile_pool(name="wstage", bufs=1) as wst:
            w1_f32 = wst.tile([P, G, H], F32)
            w2_f32 = wst.tile([P, MH, C], F32)
            nc.scalar.dma_start(out=w1_f32, in_=w1tr)
            nc.scalar.dma_start(out=w2_f32, in_=w2tr)
            nc.vector.tensor_copy(out=w1_sb, in_=w1_f32)
            nc.vector.tensor_copy(out=w2_sb, in_=w2_f32)
            nc.scalar.dma_start(out=b1_sb, in_=b1r)
            nc.scalar.dma_start(out=b2_sb, in_=b2r)
            nc.scalar.dma_start(out=mw1_sb, in_=mw1tr)
            nc.scalar.dma_start(out=mw2_sb, in_=mw2tr)
            nc.scalar.dma_start(out=mb1_sb, in_=mb1r)
            nc.scalar.dma_start(out=mb2_sb, in_=mb2r)

        # ---- phases B (c: gate matmuls) and A (x: stats), interleaved ----
        with ExitStack() as ab:
            cpool = ab.enter_context(tc.tile_pool(name="cpool", bufs=3))
            work = ab.enter_context(tc.tile_pool(name="work", bufs=2))
            ps_h = ab.enter_context(tc.tile_pool(name="ps_h", bufs=2, space="PSUM"))
            ps_g = ab.enter_context(tc.tile_pool(name="ps_g", bufs=2, space="PSUM"))

            def emit_a(i):
                g, ti = divmod(i, nt_a)
                sl = slice(ti * lt_a, (ti + 1) * lt_a)
                # casting DMA (SWDGE): f32 DRAM -> fp16 resident SBUF in one
                # step; no staging tile, no ACT copy.
                nc.gpsimd.dma_start(out=xres[:, g, sl], in_=xr[:, g, sl])
                xv = xres[:, g, sl].rearrange("p (s f) -> p s f", f=512)
                for s in range(lt_a // 512):
                    nc.vector.bn_stats(
                        out=stats_parts[:, g, ti, s], in_=xv[:, s]
                    )

            def emit_front(t):
                # cast + mm1 + relu for tile t
                sl = slice(t * lt_b, (t + 1) * lt_b)
                ct = cpool.tile([P, G, lt_b], F32, tag="c")
                nc.sync.dma_start(out=ct, in_=cr[:, :, sl])
                cbf = work.tile([P, G, lt_b], FP8, tag="cbf", bufs=1)
                for g in range(G):
                    nc.vector.tensor_scalar(
                        out=cbf[:, g], in0=ct[:, g],
                        scalar1=1.0, scalar2=None,
                        op0=mybir.AluOpType.mult,
                        op1=mybir.AluOpType.add,
                        accum_out=cond_parts[:, g, t : t + 1],
                    )
                hbf = work.tile([P, MH, lt_b], FP8, tag="hbf", bufs=1)
                for m in range(MH):
                    hps = ps_h.tile([P, lt_b], F32, tag="hps")
                    for n in range(lt_b // 512):
                        nsl = slice(n * 512, (n + 1) * 512)
                        for kk in range(0, G, 2):
                            nc.tensor.matmul(
                                hps[:, nsl],
                                lhsT=w1_sb[:, kk : kk + 2, m * P : (m + 1) * P],
                                rhs=cbf[:, kk : kk + 2, nsl],
                                start=(kk == 0), stop=(kk == G - 2),
                                perf_mode=mybir.MatmulPerfMode.DoubleRow,
                            )
                    nc.scalar.activation(
                        out=hbf[:, m], in_=hps,
                        func=mybir.ActivationFunctionType.Relu,
                        bias=b1_sb[:, m : m + 1], scale=1.0,
                    )
                return hbf

            def emit_back(t, hbf):
                # mm2 + sigmoid for tile t
                for g in range(G):
                    gps = ps_g.tile([P, lt_b], F32, tag="gps")
                    for n in range(lt_b // 512):
                        nsl = slice(n * 512, (n + 1) * 512)
                        nc.tensor.matmul(
                            gps[:, nsl],
                            lhsT=w2_sb[:, 0:MH, g * P : (g + 1) * P],
                            rhs=hbf[:, 0:MH, nsl],
                            start=True, stop=True,
                            perf_mode=mybir.MatmulPerfMode.DoubleRow,
                        )
                    gscr = work.tile([P, lt_b], FP8, tag="gscr", bufs=1)
                    nc.scalar.activation(
                        out=gscr, in_=gps,
                        func=mybir.ActivationFunctionType.Sigmoid,
                        bias=b2_sb[:, g : g + 1], scale=1.0,
                        accum_out=gmix_parts[:, g, t : t + 1],
                    )

            # Phases B and A interleaved so neither queues fully behind
            # the other (priority follows emission order).
            n_x_tiles = G * nt_a
            for t in range(nt_b):
                hbf = emit_front(t)
                emit_back(t, hbf)
                for i in range(
                    t * n_x_tiles // nt_b, (t + 1) * n_x_tiles // nt_b
                ):
                    emit_a(i)

        # ---- stats aggregation + tiny MLP + per-channel affine params ----
        with tc.tile_pool(name="ps_t", bufs=1, space="PSUM") as ps_t:
            # per-channel mean/var
            for g in range(G):
                nc.vector.bn_aggr(out=mv[:, g], in_=stats_parts[:, g])

            # layer stats: reduce over all 512 channels with a ones-matmul
            nc.vector.tensor_copy(out=vec_sb[:, :, 0:2], in_=mv)
            nc.vector.tensor_mul(
                out=vec_sb[:, :, 2], in0=mv[:, :, 0], in1=mv[:, :, 0]
            )
            lps = ps_t.tile([1, G * 3], F32, tag="lps")
            nc.tensor.matmul(
                lps, lhsT=ones_sb, rhs=vec_sb.rearrange("p g k -> p (g k)"),
                start=True, stop=True,
            )
            l_sb = big.tile([1, G * 3], F32)
            nc.vector.tensor_copy(out=l_sb, in_=lps)
            nc.vector.reduce_sum(
                out=sums3,
                in_=l_sb.rearrange("p (g k) -> p k g", g=G),
                axis=mybir.AxisListType.X,
            )
            # mu_l = S_mu/C ; var_l = (S_var + S_mu2)/C - mu_l^2
            nc.vector.tensor_add(
                out=lscr[:, 0:1], in0=sums3[:, 1:2], in1=sums3[:, 2:3]
            )
            nc.vector.tensor_scalar_mul(
                out=lscr[:, 0:1], in0=lscr[:, 0:1], scalar1=1.0 / C
            )
            nc.vector.tensor_scalar_mul(
                out=lscr[:, 1:2], in0=sums3[:, 0:1], scalar1=1.0 / C
            )
            nc.vector.tensor_mul(
                out=lscr[:, 2:3], in0=lscr[:, 1:2], in1=lscr[:, 1:2]
            )
            nc.vector.tensor_sub(
                out=lscr[:, 3:4], in0=lscr[:, 0:1], in1=lscr[:, 2:3]
            )
            nc.scalar.activation(
                out=musig_l[:, 1:2], in_=lscr[:, 3:4],
                func=mybir.ActivationFunctionType.Sqrt,
                bias=eps_sb[0:1], scale=1.0,
            )
            nc.vector.tensor_copy(out=musig_l[:, 0:1], in_=lscr[:, 1:2])
            nc.gpsimd.partition_broadcast(musig_b, musig_l)

            # sigma_c = sqrt(var_c + eps)
            nc.scalar.activation(
                out=sig_c, in_=mv[:, :, 1],
                func=mybir.ActivationFunctionType.Sqrt,
                bias=eps_sb, scale=1.0,
            )
            # g_mix, cond
            nc.vector.reduce_sum(out=gmix, in_=gmix_parts, axis=mybir.AxisListType.X)
            nc.vector.tensor_scalar_mul(out=gmix, in0=gmix, scalar1=1.0 / L)
            nc.vector.reduce_sum(out=cond, in_=cond_parts, axis=mybir.AxisListType.X)
            nc.vector.tensor_scalar_mul(out=cond, in0=cond, scalar1=1.0 / L)

            # tiny MLP: gamma/beta = mw2 @ relu(mw1 @ cond + mb1) + mb2
            h2ps = ps_t.tile([P, MH], F32, tag="h2ps")
            for m in range(MH):
                for k in range(G):
                    nc.tensor.matmul(
                        h2ps[:, m : m + 1],
                        lhsT=mw1_sb[:, k, m * P : (m + 1) * P],
                        rhs=cond[:, k : k + 1],
                        start=(k == 0), stop=(k == G - 1),
                    )
            for m in range(MH):
                nc.scalar.activation(
                    out=h2_sb[:, m : m + 1], in_=h2ps[:, m : m + 1],
                    func=mybir.ActivationFunctionType.Relu,
                    bias=mb1_sb[:, m : m + 1], scale=1.0,
                )
            gbps = ps_t.tile([P, 2 * G], F32, tag="gbps")
            for mg in range(2 * G):
                for k in range(MH):
                    nc.tensor.matmul(
                        gbps[:, mg : mg + 1],
                        lhsT=mw2_sb[:, k, mg * P : (mg + 1) * P],
                        rhs=h2_sb[:, k : k + 1],
                        start=(k == 0), stop=(k == MH - 1),
                    )
            nc.vector.tensor_add(out=gb_sb, in0=gbps, in1=mb2_sb)

            # mu = mu_l + gmix*(mu_c - mu_l); sigma = sig_l + gmix*(sig_c - sig_l)
            mu_t = big.tile([P, G], F32)
            sg_t = big.tile([P, G], F32)
            nc.vector.tensor_scalar(
                out=mu_t, in0=mv[:, :, 0], scalar1=musig_b[:, 0:1], scalar2=None,
                op0=mybir.AluOpType.subtract,
            )
            nc.vector.tensor_mul(out=mu_t, in0=mu_t, in1=gmix)
            nc.vector.tensor_scalar(
                out=mu_t, in0=mu_t, scalar1=musig_b[:, 0:1], scalar2=None,
                op0=mybir.AluOpType.add,
            )
            nc.vector.tensor_scalar(
                out=sg_t, in0=sig_c, scalar1=musig_b[:, 1:2], scalar2=None,
                op0=mybir.AluOpType.subtract,
            )
            nc.vector.tensor_mul(out=sg_t, in0=sg_t, in1=gmix)
            nc.vector.tensor_scalar(
                out=sg_t, in0=sg_t, scalar1=musig_b[:, 1:2], scalar2=None,
                op0=mybir.AluOpType.add,
            )
            # a = (1+gamma)/sigma ; b = beta - mu*a
            rs_t = big.tile([P, G], F32)
            nc.vector.reciprocal(out=rs_t, in_=sg_t)
            nc.vector.tensor_scalar(
                out=a_sb, in0=gb_sb[:, 0:G], scalar1=1.0, scalar2=None,
                op0=mybir.AluOpType.add,
            )
            nc.vector.tensor_mul(out=a_sb, in0=a_sb, in1=rs_t)
            nc.vector.tensor_mul(out=bb_sb, in0=mu_t, in1=a_sb)
            nc.vector.tensor_sub(out=bb_sb, in0=gb_sb[:, G : 2 * G], in1=bb_sb)

        # ---- phase C: imp = sum |a*x + b| over L, from resident fp16 x ----
        with ExitStack() as cs:
            scp = cs.enter_context(tc.tile_pool(name="scp", bufs=2))
            ps_m = cs.enter_context(tc.tile_pool(name="ps_m", bufs=1, space="PSUM"))
            ident_sb = scp.tile([P, P], F32, bufs=1)
            impF = scp.tile([1, C], F32, bufs=1)
            impB = scp.tile([P, C], F32, bufs=1)
            make_identity(nc, ident_sb)
            # Split by measured per-unit cost: ACT (Abs+accum) ~7.3us,
            # DVE (affine + abs-reduce) ~10.9us -> 5 : 3 of 8 units.
            for u, (t, g) in enumerate(
                (t, g) for t in range(nt_c) for g in range(G)
            ):
                sl = slice(t * lt_c, (t + 1) * lt_c)
                scr = scp.tile([P, lt_c], F16, tag="scr")
                if u % 8 in (0, 2, 3, 5, 7):
                    # ACT: imp accum via |a*x + b| activation
                    nc.scalar.activation(
                        out=scr, in_=xres[:, g, sl],
                        func=mybir.ActivationFunctionType.Abs,
                        bias=bb_sb[:, g : g + 1], scale=a_sb[:, g : g + 1],
                        accum_out=imp_parts[:, g, t : t + 1],
                    )
                else:
                    # DVE: affine then abs-reduce (offloads ACT)
                    nc.vector.tensor_scalar(
                        out=scr, in0=xres[:, g, sl],
                        scalar1=a_sb[:, g : g + 1],
                        scalar2=bb_sb[:, g : g + 1],
                        op0=mybir.AluOpType.mult, op1=mybir.AluOpType.add,
                    )
                    nc.vector.tensor_reduce(
                        out=imp_parts[:, g, t : t + 1], in_=scr,
                        axis=mybir.AxisListType.X, op=mybir.AluOpType.add,
                        apply_absolute_value=True,
                    )
            nc.vector.reduce_sum(out=imp, in_=imp_parts, axis=mybir.AxisListType.X)

            # rank_i = #{j : imp_j > imp_i};  mask = rank < KEEP
            tps = ps_m.tile([1, C], F32, tag="tps")
            for g in range(G):
                nc.tensor.transpose(
                    tps[:, g * P : (g + 1) * P], imp[:, g : g + 1], ident_sb
                )
            nc.vector.tensor_copy(out=impF, in_=tps)
            nc.gpsimd.partition_broadcast(impB, impF)
            cscr = scp.tile([P, C], F32, tag="cscr")
            for g in range(G):
                nc.vector.tensor_scalar(
                    out=cscr, in0=impB,
                    scalar1=imp[:, g : g + 1], scalar2=None,
                    op0=mybir.AluOpType.is_gt,
                    op1=mybir.AluOpType.add,
                    accum_out=rank[:, g : g + 1],
                )
            nc.vector.tensor_scalar(
                out=mask, in0=rank, scalar1=float(KEEP), scalar2=None,
                op0=mybir.AluOpType.is_lt,
            )
            nc.vector.tensor_mul(out=a_sb, in0=a_sb, in1=mask)
            nc.vector.tensor_mul(out=bb_sb, in0=bb_sb, in1=mask)

        # ---- phase D: y = a'*x + b' ----
        with tc.tile_pool(name="od", bufs=3) as od:
            for i, (g, t) in enumerate(
                (g, t) for t in range(nt_d) for g in range(G)
            ):
                sl = slice(t * lt_d, (t + 1) * lt_d)
                ot = od.tile([P, lt_d], F32, tag="ot")
                nc.vector.tensor_scalar(
                    out=ot, in0=xres[:, g, sl],
                    scalar1=a_sb[:, g : g + 1], scalar2=bb_sb[:, g : g + 1],
                    op0=mybir.AluOpType.mult, op1=mybir.AluOpType.add,
                )
                # alternate between the two HWDGE rings
                ring = nc.sync if i % 2 == 0 else nc.scalar
                ring.dma_start(out=yr[:, g, sl], in_=ot)

    nc.compile()
    return nc


_CACHED = {}


def _get_program(L=16384):
    if L not in _CACHED:
        _CACHED[L] = build_program(L=L)
    return _CACHED[L]


def make_in_maps(inputs, n_cores=B):
    f = np.float32
    w1t = np.ascontiguousarray(np.asarray(inputs["gate_w1"], dtype=f).T)
    w2t = np.ascontiguousarray(np.asarray(inputs["gate_w2"], dtype=f).T)
    mw1t = np.ascontiguousarray(np.asarray(inputs["mlp_w1"], dtype=f).T)
    mw2t = np.ascontiguousarray(np.asarray(inputs["mlp_w2"], dtype=f).T)
    b1 = np.ascontiguousarray(np.asarray(inputs["gate_b1"], dtype=f))
    b2 = np.ascontiguousarray(np.asarray(inputs["gate_b2"], dtype=f))
    mb1 = np.ascontiguousarray(np.asarray(inputs["mlp_b1"], dtype=f))
    mb2 = np.ascontiguousarray(np.asarray(inputs["mlp_b2"], dtype=f))
    x = np.asarray(inputs["x16"], dtype=f)
    c = np.asarray(inputs["c16"], dtype=f)
    maps = []
    for b in range(n_cores):
        maps.append({
            "x": np.ascontiguousarray(x[b]),
            "c": np.ascontiguousarray(c[b]),
            "w1t": w1t, "b1": b1, "w2t": w2t, "b2": b2,
            "mw1t": mw1t, "mb1": mb1, "mw2t": mw2t, "mb2": mb2,
        })
    return maps


def run(inputs, trace=False, **kw):
    """Run on 8 cores; returns (output [8,C,L], BassKernelResults)."""
    nc = _get_program()
    in_maps = make_in_maps(inputs)
    res = run_bass_kernel_spmd(nc, in_maps, list(range(B)), trace=trace, **kw)
    out = np.stack([res.results[i]["y"] for i in range(B)], axis=0)
    return out, res


def kernel(**inputs):
    out, _ = run(inputs)
    return out.astype(np.asarray(inputs["x16"]).dtype, copy=False)
